# revision 1
# baseline (speedup 1.0000x reference)
"""Multi-head self-attention (B=4, S=2048, D=1024, H=16, Hd=64) on 8 TRN2 cores.

Sharding: core c -> (batch b = c//2, head-group g = c%2 of 8 heads).
Each core computes its batch's 8 heads end-to-end plus the partial output
projection for its head group; the host sums the two head-group partials
per batch. No collectives.

v3: fp8e4 DoubleRow matmuls (0.5 cycles/row) for scores, attn@V, and the
softmax denominator; softmax normalization via DVE reciprocal + a constant
E-matrix broadcast matmul. The whole kernel is one flat software-pipelined
schedule: Q/K projections are emitted as per-512-column blocks and V / later
head-pairs' projections / the output projection are interleaved between
attention units as background chunks, so the Activation engine (exp) and PE
stay concurrently busy instead of phase-alternating.
"""

from contextlib import ExitStack

import numpy as np
import ml_dtypes

import concourse.bass as bass
import concourse.tile as tile
from concourse import mybir
from concourse.bass_utils import run_bass_kernel_spmd
from concourse.vector_clock import ScopedClock
from bass_rust import InstNoOp, SyncInfo

BF16 = mybir.dt.bfloat16
F32 = mybir.dt.float32
FP8 = mybir.dt.float8e4
AF = mybir.ActivationFunctionType
DR = mybir.MatmulPerfMode.DoubleRow

B, S, D = 4, 2048, 1024
H, HD = 16, 64
GH = 8          # heads per core
GM = GH * HD    # 512 head dims per core
NQB = 4         # q blocks of 512
QB = 512
NKC = 16        # k chunks of 128
NJ = NKC // 2   # k chunk pairs
NDC = 8         # d chunks of 128 (contraction for projections)
NMT = GM // 128  # 4 m-tiles for projections

_META_TYPES = ("TileBranchInst", "BassTileLoopBlock", "BassTilePoolBoundary")


class _TileCtx(tile.TileContext):
    """Splits multi-sem-wait instructions: the pinned walrus rejects any TPB
    instruction carrying more than one sem-wait, while Tile emits joins and a
    global end-of-context drain with several."""

    def _split_waits(self, ordered):
        nc = self.nc
        for bb_name, insts in ordered.items():
            out = []
            for inst in insts:
                si = inst.sync_info
                if (
                    si is not None
                    and si.on_wait
                    and len(si.on_wait) > 1
                    and type(inst).__name__ not in _META_TYPES
                    and inst.engine != mybir.EngineType.Unassigned
                ):
                    waits = list(si.on_wait)
                    for w in waits[:-1]:
                        nop = InstNoOp(
                            name=nc.get_next_instruction_name(), ins=[], outs=[]
                        )
                        nop.engine = inst.engine
                        nop.sync_info = SyncInfo(on_wait=[w], on_update=[])
                        out.append(nop)
                    inst.sync_info = SyncInfo(
                        on_wait=[waits[-1]], on_update=list(si.on_update)
                    )
                out.append(inst)
            ordered[bb_name] = out

    def _lower_ordered_insts(self, ordered):
        self._split_waits(ordered)
        super()._lower_ordered_insts(ordered)

    def _drain_and_barrier(self, tick_clock, wait_clock):
        drain_inst = self.nc.sync.drain()
        wait_clock.add_sem_waits(
            drain_inst.ins, ScopedClock({None: tick_clock.global_clock})
        )
        si = drain_inst.ins.sync_info
        waits = list(si.on_wait) if si is not None else []
        if len(waits) > 1:
            drain_inst.ins.sync_info = SyncInfo(
                on_wait=waits[:1], on_update=list(si.on_update)
            )
            for w in waits[1:]:
                extra = self.nc.sync.drain()
                extra.ins.sync_info = SyncInfo(on_wait=[w], on_update=[])

        self.nc.all_engine_barrier()
        assert self.sems is not None
        popped = self.nc._tile_sem_poison_stack.pop()
        assert popped is self._sem_poison
        self.nc.clear_and_free_semaphores(list(self.sems.allocated().values()))
        self.nc.all_engine_barrier()


def _build_program():
    nc = bass.Bass(trn_type="TRN2", debug=False, num_devices=8)

    xT = nc.dram_tensor("xT", [D, S], BF16, kind="ExternalInput").ap()
    wq = nc.dram_tensor("wq", [D, GM], BF16, kind="ExternalInput").ap()
    wk = nc.dram_tensor("wk", [D, GM], BF16, kind="ExternalInput").ap()
    wv = nc.dram_tensor("wv", [D, GM], BF16, kind="ExternalInput").ap()
    # pair-major-reordered Wo.T slice: [128, 4 pairs x 1024]
    wo = nc.dram_tensor("wo", [128, NMT * D], BF16, kind="ExternalInput").ap()
    bq = nc.dram_tensor("bq", [GM], F32, kind="ExternalInput").ap()
    bk = nc.dram_tensor("bk", [GM], F32, kind="ExternalInput").ap()
    bo = nc.dram_tensor("bo", [D], F32, kind="ExternalInput").ap()
    outT = nc.dram_tensor("outT", [D, S], F32, kind="ExternalOutput").ap()

    with _TileCtx(nc) as tc, ExitStack() as ctx:
        const_pool = ctx.enter_context(tc.tile_pool(name="const", bufs=1))
        act_pool = ctx.enter_context(tc.tile_pool(name="acts", bufs=1))
        qk8_pool = ctx.enter_context(tc.tile_pool(name="qk8", bufs=2))
        slab_pool = ctx.enter_context(tc.tile_pool(name="slab", bufs=32))
        y_pool = ctx.enter_context(tc.tile_pool(name="y", bufs=4))
        s_ps = ctx.enter_context(tc.tile_pool(name="s_ps", bufs=2, space="PSUM"))
        o_ps = ctx.enter_context(tc.tile_pool(name="o_ps", bufs=1, space="PSUM"))
        d_ps = ctx.enter_context(tc.tile_pool(name="d_ps", bufs=1, space="PSUM"))
        sm_ps = ctx.enter_context(tc.tile_pool(name="sm_ps", bufs=2, space="PSUM"))

        # ---- weights / inputs / constants --------------------------------
        # DMA order is tuned so the first K/Q column block (and then V) can
        # start as early as possible: biases + mi=0 weight slices + x column
        # block 0 first, then wv, then the rest.
        bq_sb = const_pool.tile([128, NMT], F32, tag="bq")
        nc.sync.dma_start(bq_sb[:], bq.rearrange("(c p) -> p c", p=128))
        bk_sb = const_pool.tile([128, NMT], F32, tag="bk")
        nc.sync.dma_start(bk_sb[:], bk.rearrange("(c p) -> p c", p=128))

        wk_sb = const_pool.tile([128, NDC * GM], BF16, tag="wk")
        wq_sb = const_pool.tile([128, NDC * GM], BF16, tag="wq")
        for wsb, src in ((wk_sb, wk), (wq_sb, wq)):
            nc.sync.dma_start(
                wsb[:].rearrange("p (c m) -> p c m", m=GM)[:, :, 0:128],
                src.rearrange("(c p) m -> p c m", p=128)[:, :, 0:128],
            )
        xt = const_pool.tile([128, NDC * S], BF16, tag="xt")
        for dc in range(NDC):
            nc.sync.dma_start(
                xt[:, dc * S : dc * S + QB],
                xT[dc * 128 : (dc + 1) * 128, 0:QB],
            )
        nc.sync.dma_start(
            xt[:].rearrange("p (c s) -> p c s", s=S)[:, :, QB:S],
            xT.rearrange("(c p) s -> p c s", p=128)[:, :, QB:S],
        )
        wv_sb = const_pool.tile([128, NDC * GM], BF16, tag="wv")
        wo_sb = const_pool.tile([128, NMT * D], BF16, tag="wo")
        bo_sb = const_pool.tile([128, NDC], F32, tag="bo")

        def load_wv():
            nc.sync.dma_start(
                wv_sb[:].rearrange("p (c m) -> p c m", m=GM),
                wv.rearrange("(c p) m -> p c m", p=128),
            )

        def load_wkq_rest():
            for wsb, src in ((wk_sb, wk), (wq_sb, wq)):
                nc.sync.dma_start(
                    wsb[:].rearrange("p (c m) -> p c m", m=GM)[:, :, 128:GM],
                    src.rearrange("(c p) m -> p c m", p=128)[:, :, 128:GM],
                )

        def load_wo():
            nc.sync.dma_start(wo_sb[:], wo[:, :])
            nc.sync.dma_start(bo_sb[:], bo.rearrange("(c p) -> p c", p=128))

        ones8 = const_pool.tile([128, 2 * 32], FP8, tag="ones8")
        nc.vector.memset(ones8[:], 1.0)
        ones64 = const_pool.tile([1, 64], BF16, tag="ones64")
        nc.vector.memset(ones64[:], 1.0)
        rd = [
            act_pool.tile([1, QB], BF16, name=f"rd{i}", tag=f"rd{i}")
            for i in range(2)
        ]
        bc_sb = [
            act_pool.tile([64, QB], F32, name=f"bcs{i}", tag=f"bcs{i}")
            for i in range(2)
        ]
        tmp_o = [
            act_pool.tile([64, QB], BF16, name=f"tmpo{i}", tag=f"tmpo{i}")
            for i in range(2)
        ]

        # ---- persistent activations -------------------------------------
        # qtr/ktr: [32, 8 heads x (2 pair x 2048 seq)] fp8, pair = hd d vs d+32
        qtr = act_pool.tile([32, GH * 2 * S], FP8, tag="qtr")
        ktr = act_pool.tile([32, GH * 2 * S], FP8, tag="ktr")
        # v8: [128 keys, 8 jpairs x (8 heads x 2 chunk x 64 dim)] fp8
        v8 = act_pool.tile([128, NKC * 512], FP8, tag="v8")
        # O^T per head-pair: [128, S] bf16; even head rows 0-63, odd 64-127.
        otp = [
            act_pool.tile([128, S], BF16, name=f"otp{t}", tag=f"otp{t}")
            for t in range(NMT)
        ]

        # ---- background task closures ------------------------------------
        # Warm the PE p-state so the first projection chains run at full
        # clock: a stream of tiny matmuls on constant data.
        wps = sm_ps.tile([64, 64], F32, name="wps", tag="sm")
        for i in range(56):
            nc.tensor.matmul(
                wps[:], ones64[:], ones64[:], start=True, stop=True,
            )

        def kq_col(w_sb, b_sb, dst, mi, c, head=False):
            """One 512-col block of a Q/K projection m-tile: matmul chain +
            bias-add to fp8 + partition rearrange to the pair-split layout."""
            ps = sm_ps.tile([128, QB], F32, name="kqp", tag="sm")
            for dc in range(NDC):
                nc.tensor.matmul(
                    ps[:],
                    w_sb[:, dc * GM + mi * 128 : dc * GM + (mi + 1) * 128],
                    xt[:, dc * S + c * QB : dc * S + (c + 1) * QB],
                    start=(dc == 0),
                    stop=(dc == NDC - 1),
                )
            q8 = qk8_pool.tile([128, QB], FP8, name="q8blk", tag="q8")
            nc.vector.tensor_scalar_add(q8[:], ps[:], b_sb[:, mi : mi + 1])
            # head blocks rearrange on the idle Activation DGE queue so they
            # don't wait behind the bulk loads on the SP queue.
            dma = nc.scalar.dma_start if head else nc.sync.dma_start
            for hh in range(2):
                for i in range(2):
                    h = 2 * mi + hh
                    dma(
                        dst[
                            :,
                            h * 2 * S + i * S + c * QB : h * 2 * S + i * S + (c + 1) * QB,
                        ],
                        q8[hh * 64 + i * 32 : hh * 64 + i * 32 + 32, :],
                    )

        def v_proj(si, t):
            # V projection for keys chunk si, head-pair t only (N=128 cols).
            j, par = si // 2, si % 2
            ps = sm_ps.tile([128, 128], F32, name="vp", tag="sm")
            for dc in range(NDC):
                nc.tensor.matmul(
                    ps[:],
                    xt[:, dc * S + si * 128 : dc * S + (si + 1) * 128],
                    wv_sb[:, dc * GM + t * 128 : dc * GM + (t + 1) * 128],
                    start=(dc == 0),
                    stop=(dc == NDC - 1),
                )
            # psum [128, (h 2)(m 64)] -> v8[:, j*1024 + (2t+hh)*128 + par*64 + m]
            nc.vector.tensor_copy(
                v8[:]
                .rearrange("p (j h i m) -> p j h i m", h=GH, i=2, m=HD)[
                    :, j : j + 1, 2 * t : 2 * t + 2, par : par + 1, :
                ],
                ps[:].rearrange("p (h m) -> p h m", h=2),
            )

        def out_proj_cols(qb, ecs):
            for ec in ecs:
                ps = sm_ps.tile([128, QB], F32, name="yp", tag="sm")
                for mt in range(NMT):
                    nc.tensor.matmul(
                        ps[:],
                        wo_sb[:, mt * D + ec * 128 : mt * D + (ec + 1) * 128],
                        otp[mt][:, qb * QB : (qb + 1) * QB],
                        start=(mt == 0),
                        stop=(mt == NMT - 1),
                    )
                y_sb = y_pool.tile([128, QB], F32, name="yblk", tag="y")
                nc.vector.tensor_scalar_add(y_sb[:], ps[:], bo_sb[:, ec : ec + 1])
                nc.sync.dma_start(
                    outT[ec * 128 : (ec + 1) * 128, qb * QB : (qb + 1) * QB], y_sb[:]
                )

        # ---- attention unit emitters -------------------------------------
        # DVE fast-exp: e4m3 bits of exp(s*0.125) are an affine function of s
        # (linear-mantissa log2 trick): bits = s*0.125*8/ln2 + 56 + c.
        FEXP_A = 0.125 * 8.0 / float(np.log(2.0))
        FEXP_B = 55.55  # HW convert rounds-to-nearest; Schraudolph-centered
        FEXP_J = (3, 6)   # kc-pairs whose exp runs on DVE instead of Act

        def att_scores(t, qb, hh, j):
            """Scores + exp for head 2t+hh at (qb, kc-pair j)."""
            h = 2 * t + hh
            with tc.high_priority(offset=300):
                s2 = s_ps.tile([128, 2 * QB], F32, name="s2", tag="s2")
                qv = qtr[:, h * 2 * S : (h + 1) * 2 * S].rearrange(
                    "p (i s) -> p i s", i=2
                )[:, :, qb * QB : (qb + 1) * QB]
                for par in range(2):
                    kc = 2 * j + par
                    kv = ktr[:, h * 2 * S : (h + 1) * 2 * S].rearrange(
                        "p (i s) -> p i s", i=2
                    )[:, :, kc * 128 : (kc + 1) * 128]
                    nc.tensor.matmul(
                        s2[:, par * QB : (par + 1) * QB],
                        kv, qv, start=True, stop=True, perf_mode=DR,
                    )
                sl = slab_pool.tile([128, 2 * QB], FP8, name="sl", tag="slab")
                if j in FEXP_J:
                    nc.vector.tensor_scalar(
                        sl[:].bitcast(mybir.dt.uint8),
                        s2[:],
                        FEXP_A,
                        FEXP_B,
                        mybir.AluOpType.mult,
                        mybir.AluOpType.add,
                    )
                else:
                    nc.scalar.activation(sl[:], s2[:], AF.Exp, scale=0.125)
            return sl

        state = {"po": None, "dn": None, "n": 0}

        def att_consume(t, qb, hh, j, sl):
            """attn@V + denominator accumulation for unit (t, qb, hh, j)."""
            h = 2 * t + hh
            if j == 0:
                state["po"] = o_ps.tile([64, QB], F32, name="po", tag="po")
                state["dn"] = d_ps.tile([32, QB], F32, name="dn", tag="dn")
            po, dn = state["po"], state["dn"]
            rhs = sl[:].rearrange("p (i q) -> p i q", i=2)
            lhsT = v8[
                :, j * 1024 + h * 128 : j * 1024 + (h + 1) * 128
            ].rearrange("p (i m) -> p i m", i=2)
            nc.tensor.matmul(
                po[:, :], lhsT, rhs,
                start=(j == 0), stop=(j == NJ - 1), perf_mode=DR,
            )
            nc.tensor.matmul(
                dn[:, :],
                ones8[:].rearrange("p (i m) -> p i m", i=2),
                rhs,
                start=(j == 0), stop=(j == NJ - 1), perf_mode=DR,
            )
            if j == NJ - 1:
                k = state["n"] % 2
                state["n"] += 1
                r = rd[k]
                with nc.allow_low_precision(reason="softmax recip in bf16"):
                    nc.vector.reciprocal(r[0:1, :], dn[0:1, :])
                bc = sm_ps.tile([64, QB], F32, name="bc", tag="sm")
                nc.tensor.matmul(bc[:], ones64[:], r[:], start=True, stop=True)
                bcs = bc_sb[k]
                nc.vector.tensor_copy(bcs[:], bc[:])
                if hh == 0:
                    nc.vector.tensor_mul(
                        otp[t][0:64, qb * QB : (qb + 1) * QB], po[:], bcs[:]
                    )
                else:
                    tm = tmp_o[k]
                    nc.vector.tensor_mul(tm[:], po[:], bcs[:])
                    nc.sync.dma_start(
                        otp[t][64:128, qb * QB : (qb + 1) * QB], tm[:]
                    )
                    if t == NMT - 1:
                        out_proj_cols(qb, range(0, 4))
                        out_proj_cols(qb, range(4, 8))

        # ---- the flat schedule -------------------------------------------
        kq_col(wk_sb, bk_sb, ktr, 0, 0, head=True)
        kq_col(wq_sb, bq_sb, qtr, 0, 0, head=True)
        load_wv()

        bg = {}

        def bg_add(u, fn, *args):
            bg.setdefault(u, []).append((fn, args))

        # V for head-pair t: needed from unit 64*t (its qb0-h0-j0 consume).
        # Emit chunk (si pair j) no later than iteration j of that window
        # (consume of unit u happens at iteration u+1).
        bg_add(0, v_proj, 0, 0)
        bg_add(0, v_proj, 1, 0)
        for j in range(1, NJ):
            bg_add(j, v_proj, 2 * j, 0)
            bg_add(j, v_proj, 2 * j + 1, 0)
        for t in range(1, NMT):
            base = 64 * (t - 1) + 16
            for si in range(NKC):
                bg_add(base + 2 * si, v_proj, si, t)
        # remaining K0/Q0 column blocks: scores at iteration j read K cols
        # kc=2j,2j+1 (c-block j//2), so K0c_c must be emitted by bg(2c-1).
        for i, c in enumerate((1, 2, 3)):
            bg_add(2 * c - 1, kq_col, wk_sb, bk_sb, ktr, 0, c)
            bg_add(2 * c, kq_col, wq_sb, bq_sb, qtr, 0, c)
        # bulk loads, positioned in the SP queue after the early rearranges
        bg_add(8, load_wkq_rest)
        bg_add(24, load_wo)
        # K/Q projections for head-pair mi: spread over window mi-1
        for mi in range(1, NMT):
            base = 64 * (mi - 1) + 9
            for c in range(4):
                bg_add(base + 8 * c, kq_col, wk_sb, bk_sb, ktr, mi, c)
                bg_add(base + 8 * c + 4, kq_col, wq_sb, bq_sb, qtr, mi, c)

        units = [
            (t, qb, hh, j)
            for t in range(NMT)
            for qb in range(NQB)
            for hh in range(2)
            for j in range(NJ)
        ]
        pending = None
        for u in range(len(units) + 1):
            if u < len(units):
                t, qb, hh, j = units[u]
                sl = att_scores(t, qb, hh, j)
            for fn, args in bg.get(u, []):
                fn(*args)
            if pending is not None:
                att_consume(*pending)
            pending = (t, qb, hh, j, sl) if u < len(units) else None
        # (loop epilogue consumed the final unit via the +1 iteration)

    return nc


_NC = None
_last_in_maps = None


def _get_program():
    global _NC
    if _NC is None:
        _NC = _build_program()
    return _NC


def kernel(x, Wq, bq, Wk, bk, Wv, bv, Wo, bo):
    x = np.asarray(x, np.float32)
    bf = ml_dtypes.bfloat16
    in_maps = []
    for c in range(8):
        b, g = c // 2, c % 2
        sl = slice(g * GM, (g + 1) * GM)
        wo_slice = np.asarray(Wo, np.float32)[:, sl].T  # [512, 1024]
        # fold bv and half of bo into the output bias
        bo_eff = np.asarray(bo, np.float32) / 2.0 + np.asarray(bv, np.float32)[sl] @ wo_slice
        in_maps.append(
            {
                "xT": np.ascontiguousarray(x[b].T).astype(bf),
                "wq": np.ascontiguousarray(np.asarray(Wq, np.float32)[sl, :].T).astype(bf),
                "wk": np.ascontiguousarray(np.asarray(Wk, np.float32)[sl, :].T).astype(bf),
                "wv": np.ascontiguousarray(np.asarray(Wv, np.float32)[sl, :].T).astype(bf),
                "wo": np.ascontiguousarray(
                    wo_slice.reshape(GM // 128, 128, D).transpose(1, 0, 2).reshape(128, (GM // 128) * D)
                ).astype(bf),
                "bq": np.ascontiguousarray(np.asarray(bq, np.float32)[sl]),
                "bk": np.ascontiguousarray(np.asarray(bk, np.float32)[sl]),
                "bo": np.ascontiguousarray(bo_eff.astype(np.float32)),
            }
        )

    global _last_in_maps
    _last_in_maps = in_maps
    nc = _get_program()
    res = run_bass_kernel_spmd(nc, in_maps, core_ids=list(range(8)))
    out = np.empty((B, S, D), np.float32)
    for b in range(B):
        acc = res.results[2 * b]["outT"].astype(np.float32) + res.results[
            2 * b + 1
        ]["outT"].astype(np.float32)
        out[b] = acc.T
    return out

